# revision 2
# baseline (speedup 1.0000x reference)
"""Multi-head attention Trainium2 kernel (8-core batch x head-group parallel).

Problem: B=2, S=2048, D=1024, H=16 heads x HD=64.
Sharding: core c owns batch c//4 and head-group c%4 (4 heads, J=256 columns
of Wq/Wk/Wv, 256 rows of Wo). Each core produces a partial output (its
heads' contribution through Wo); the host sums 4 partials per batch + bo.

All matmul operands are bf16 (flat 1 cycle/row on the PE; fp32r pays 4x
below N=256). PSUM accumulation stays fp32.

Per-core compute (S=2048, 4 heads as 2 head-pairs hg in {0,1}):
  K^T/Q^T  = W^T @ X^T          [128(2-head hd), 2048]  (jt = head-pair)
  V        = X @ Wv             [s, 256] orientation -> v_sb[k-tile, 4*65]
                                (65th column per head = ones, so attn@V's
                                 output column 64 = softmax denominator)
  S^T[k,q] = K_tile^T Q         head-pairs row-stacked in the PE
                                (tile_position h*64)
  P        = exp(S^T / 8)       ScalarE, scale folded in; max-subtraction
                                skipped (scores ~N(0,1))
  ao[q,hd] = sum_k P^T V        lhsT = exp tile [128k,128q] (stationary),
                                rhs = v_sb [128k, 65]: N=65 per matmul --
                                full PE-column utilization vs streaming q
  normalize: reciprocal of col 64, per-partition tensor_scalar mult (DVE)
  aoT      = transpose(ao)      fp32 PE transposes, then bf16 copy
  partial  = aoT^T @ Wo         contraction over the core's 256 head dims

Scheduling: Tile's per-engine order follows emission order. Scores groups
(hg, kt) stream on the PE with exp (ScalarE) one group behind and attn@V
six groups behind (uniform lag so the single-buffered pav psum slots
rotate without epilogue WAR stalls). Projection chains and output
(transpose + Wo) chunks are fed as filler between groups.

PSUM (8 banks): scores 2x2 (double-buffered [128,2,512]) + pavA/pavB 1+1
+ proj/wo/transpose scratch 2.
"""

import numpy as np

import concourse.bass as bass
import concourse.bacc as bacc
import concourse.tile as tile
import concourse.mybir as mybir
from concourse.masks import make_identity

F32 = mybir.dt.float32
BF16 = mybir.dt.bfloat16

P = 128


def build_nc(
    S=2048,          # sequence length (one batch per core)
    D=1024,          # model dim
    DOUT=1024,       # output dim (cols of Wo)
    J=256,           # head-dim columns per core (4 heads x 64)
    QB=512,          # q-block
    LAG=48,          # attn@V lag behind scores/exp, in groups
    exp_bufs=52,
    RATE=700,        # target filler ns per group (debt feeding)
    LOOKAHEAD=3,
    debug=False,
    prolog_feed=2,   # fillers per group while projections remain
    steady_feed=1,
):
    HD = 64
    NH = J // HD         # heads per core (4)
    NHG = NH // 2        # head-pairs (2)
    JT = J // P          # 128-wide jt chunks (2) == head-pairs
    DT = D // P          # contraction tiles for projections (8)
    NQB = S // QB        # q-blocks (4)
    KT = S // P          # key tiles (16)
    NQT = S // P         # q tiles (16)
    SCALE = 1.0 / float(np.sqrt(HD))
    assert J == 2 * P and NHG == JT
    GPQ = NHG * KT       # groups per q-block (32)

    nc = bacc.Bacc(None, target_bir_lowering=False)

    # host pre-rearranged inputs (see _prep_in_maps)
    xt_h = nc.dram_tensor("xt", [P, DT, S], BF16, kind="ExternalInput")
    wq_h = nc.dram_tensor("wq", [P, DT, J], BF16, kind="ExternalInput")
    wk_h = nc.dram_tensor("wk", [P, DT, J], BF16, kind="ExternalInput")
    wv_h = nc.dram_tensor("wv", [P, DT, J], BF16, kind="ExternalInput")
    bq_h = nc.dram_tensor("bq", [P, JT], F32, kind="ExternalInput")
    bk_h = nc.dram_tensor("bk", [P, JT], F32, kind="ExternalInput")
    bv_h = nc.dram_tensor("bv", [1, J], BF16, kind="ExternalInput")
    wo_h = nc.dram_tensor("wo", [P, JT, DOUT], BF16, kind="ExternalInput")
    out_h = nc.dram_tensor("out", [S, DOUT], F32, kind="ExternalOutput")
    if debug:
        dbg_q = nc.dram_tensor("dbg_q", [P, JT, S], BF16, kind="ExternalOutput")
        dbg_k = nc.dram_tensor("dbg_k", [P, JT, S], BF16, kind="ExternalOutput")
        dbg_v = nc.dram_tensor("dbg_v", [P, KT, J + 4], BF16, kind="ExternalOutput")
        dbg_ao = nc.dram_tensor("dbg_ao", [16, P, J], F32, kind="ExternalOutput")

    with tile.TileContext(nc) as tc:
        with (
            tc.tile_pool(name="const", bufs=1) as const,
            tc.tile_pool(name="xin", bufs=4) as xin,
            tc.tile_pool(name="expp", bufs=exp_bufs) as expp,
            tc.tile_pool(name="aop", bufs=8) as aop,
            tc.tile_pool(name="aotp", bufs=3) as aotp,
            tc.tile_pool(name="recp", bufs=2) as recp,
            tc.tile_pool(name="outp", bufs=6) as outp,
            tc.tile_pool(name="psS", bufs=2, space="PSUM") as psS,
            tc.tile_pool(name="psAV", bufs=1, space="PSUM") as psAV,
            tc.tile_pool(name="psP", bufs=2, space="PSUM") as psP,
        ):
            # ---- constants / weights (DMA order = need order: wk + biases
            # + first x half gate the first matmul; wo is needed last) ----
            wq_sb = const.tile([P, DT, J], BF16)
            wk_sb = const.tile([P, DT, J], BF16)
            wv_sb = const.tile([P, DT, J], BF16)
            wo_sb = const.tile([P, JT, DOUT], BF16)
            bq_sb = const.tile([P, JT], F32)
            bk_sb = const.tile([P, JT], F32)
            bv_row = const.tile([1, J], BF16)
            # all early-needed DMAs go on the sync queue in need-order
            # (single FIFO => device serves them in this order); only the
            # late-needed wo rides the slow gpsimd/SWDGE path


            ident_f = const.tile([P, P], F32)
            make_identity(nc, ident_f[:])
            ones_f = const.tile([P, 1], F32)
            nc.vector.memset(ones_f[:], 1.0)
            ones1_b = const.tile([1, P], BF16)
            nc.vector.tensor_copy(
                out=ones1_b[:], in_=ones_f[0:1, :].to_broadcast((1, P))
            )
            bv_bc = const.tile([P, J], BF16)

            def emit_bv_bc():
                # broadcast bv to all partitions via K=1 matmul: [128, J] bf16
                pbv = psP.tile([P, 512], F32, tag="pp", name="pp", bufs=1)
                nc.tensor.matmul(
                    pbv[:, 0:J], lhsT=ones1_b[:], rhs=bv_row[:],
                    start=True, stop=True,
                )
                nc.vector.tensor_copy(out=bv_bc[:], in_=pbv[:, 0:J])

            # ---- persistent projection outputs ----
            qT_sb = const.tile([P, JT, S], BF16)   # [hd-pair rows, jt, q]
            kT_sb = const.tile([P, JT, S], BF16)
            # v with ones columns: [k-part, kt, 4 heads x 65]
            v_sb = const.tile([P, KT, NH * (HD + 1)], BF16)
            # fill the ones columns once (static)
            ones_view = bass.AP(
                tensor=v_sb.tensor,
                offset=v_sb.offset + HD,  # first ones col at 64
                ap=[v_sb.ap[0], [NH * (HD + 1), KT], [HD + 1, NH], [1, 1]],
            )
            nc.vector.tensor_copy(
                out=ones_view,
                in_=ones_f[:].unsqueeze(1).to_broadcast((P, KT, NH, 1)),
            )

            # ---------------- projection filler units ----------------
            def dma_x(sblk):
                # two half-DMAs so the first chain can start sooner
                xh = xin.tile([P, DT, QB], BF16, tag="xh", name="xh")
                for dh in range(2):
                    w = DT // 2
                    nc.sync.dma_start(
                        xh[:, dh * w : (dh + 1) * w, :],
                        xt_h.ap()[
                            :, dh * w : (dh + 1) * w, sblk * QB : (sblk + 1) * QB
                        ],
                    )
                return xh

            NPART = 4  # chain split granularity (2 matmuls per part)

            def qk_part(box, w_sb, b_sb, dstT, xh, sblk, jt, part):
                """1/NPART of a projection chain. part 0 opens the psum
                accumulation (own tag so interleaved units can't cycle),
                the last part closes it + bias-copy."""
                if part == 0:
                    box[0] = psP.tile([P, QB], F32, tag="ppc", name="ppc", bufs=1)
                ps = box[0]
                h = DT // NPART
                for i in range(h):
                    dt_ = part * h + i
                    nc.tensor.matmul(
                        ps[:],
                        lhsT=w_sb[:, dt_, jt * P : (jt + 1) * P],
                        rhs=xh[:, dt_, :],
                        start=(dt_ == 0),
                        stop=(dt_ == DT - 1),
                    )
                if part == NPART - 1:
                    nc.vector.tensor_scalar_add(
                        out=dstT[:, jt, sblk * QB : (sblk + 1) * QB],
                        in0=ps[:],
                        scalar1=b_sb[:, jt : jt + 1],
                    )

            def qk_chain(w_sb, b_sb, dstT, xh, sblk, jt):
                box = [None]
                for part in range(NPART):
                    qk_part(box, w_sb, b_sb, dstT, xh, sblk, jt, part)

            def v_chain(xh, sblk, st):
                """V projection for one 128-seq subtile, [s, j] orientation."""
                kt = sblk * (QB // P) + st
                ps = psP.tile([P, QB], F32, tag="pp", name="pp", bufs=1)
                for dt_ in range(DT):
                    nc.tensor.matmul(
                        ps[:, 0:J],
                        lhsT=xh[:, dt_, st * P : (st + 1) * P],
                        rhs=wv_sb[:, dt_, :],
                        start=(dt_ == 0),
                        stop=(dt_ == DT - 1),
                    )
                # bias add + copy into v_sb (strided dst skips ones columns)
                vdst = bass.AP(
                    tensor=v_sb.tensor,
                    offset=v_sb.offset + kt * NH * (HD + 1),
                    ap=[v_sb.ap[0], [HD + 1, NH], [1, HD]],
                )
                nc.vector.tensor_tensor(
                    out=vdst,
                    in0=ps[:, 0:J].rearrange("p (h d) -> p h d", h=NH),
                    in1=bv_bc[:].rearrange("p (h d) -> p h d", h=NH),
                    op=mybir.AluOpType.add,
                )



            # ---------------- output-side filler units ----------------
            def trans_unit(box, ao_sb):
                """Transpose ao [128q, 256] -> aoT bf16 (into box[0])."""
                def emit():
                    aoT = aotp.tile([P, JT, P], BF16, tag="aoT", name="aoT")
                    box[0] = aoT
                    for jt in range(JT):
                        pt = psP.tile([P, QB], F32, tag="pp", name="pp", bufs=1)
                        nc.tensor.transpose(
                            pt[:, 0:P], ao_sb[:, jt * P : (jt + 1) * P], ident_f[:]
                        )
                        nc.vector.tensor_copy(out=aoT[:, jt, :], in_=pt[:, 0:P])
                return emit

            def wo_chunk(box, qt, ch, last):
                """One Wo chunk for global q-tile qt. In the tail (last
                q-block) the psum comes from the then-free scores pool (4
                banks => deeper pipelining) and the psum->sbuf copy goes to
                the then-idle ScalarE instead of DVE."""
                def emit():
                    aoT = box[0]
                    po = (
                        psS.tile([P, 2, QB], F32, tag="pss", name="pss")[:, 0, :]
                        if last
                        else psP.tile([P, QB], F32, tag="pp", name="pp", bufs=1)[:]
                    )
                    for jt in range(JT):
                        nc.tensor.matmul(
                            po,
                            lhsT=aoT[:, jt, :],
                            rhs=wo_sb[:, jt, ch * 512 : (ch + 1) * 512],
                            start=(jt == 0),
                            stop=(jt == JT - 1),
                        )
                    o_sb = outp.tile([P, 512], F32, tag="o_sb", name="o_sb")
                    if last:
                        nc.scalar.activation(
                            out=o_sb[:],
                            in_=po,
                            func=mybir.ActivationFunctionType.Copy,
                        )
                    else:
                        nc.vector.tensor_copy(out=o_sb[:], in_=po)
                    nc.sync.dma_start(
                        out_h.ap()[
                            qt * P : (qt + 1) * P, ch * 512 : (ch + 1) * 512
                        ],
                        o_sb[:],
                    )
                return emit

            # ---------------- main emission ----------------
            from collections import deque
            import heapq

            fillers = []  # heap of (deadline_group, seq, pe_cost_ns, fn)
            epiq = deque()
            fseq = [0]

            def push(deadline, cost, fn):
                heapq.heappush(fillers, (deadline, fseq[0], cost, fn))
                fseq[0] += 1

            # DMA need-order on the single sync FIFO: xh0a, wk, wq, biases,
            # xh0b, xh1, wv, bv; wo via gpsimd; xh2/xh3 deferred to fillers
            # so they can't jump ahead of early weights.
            xhs = [None] * NQB
            xh0 = xin.tile([P, DT, QB], BF16, tag="xh", name="xh")
            xhs[0] = xh0
            hw_ = DT // 2
            nc.sync.dma_start(xh0[:, 0:hw_, :], xt_h.ap()[:, 0:hw_, 0:QB])
            nc.sync.dma_start(wk_sb[:], wk_h.ap())
            nc.sync.dma_start(wq_sb[:], wq_h.ap())
            nc.sync.dma_start(xh0[:, hw_:DT, :], xt_h.ap()[:, hw_:DT, 0:QB])
            nc.sync.dma_start(bk_sb[:], bk_h.ap())
            nc.sync.dma_start(bq_sb[:], bq_h.ap())
            xhs[1] = dma_x(1)
            nc.sync.dma_start(wv_sb[:], wv_h.ap())
            nc.sync.dma_start(bv_row[:], bv_h.ap())
            nc.gpsimd.dma_start(wo_sb[:], wo_h.ap())

            # PE warm-up: burn the p-state ramp on dummy matmuls while the
            # DMAs land (results unused; input is a zeroed const tile)
            zro = const.tile([P, 512], BF16)
            nc.vector.memset(zro[:], 0.0)
            for _ in range(6):
                pw = psS.tile([P, 2, QB], F32, tag="pss", name="pss")
                for h in range(2):
                    nc.tensor.matmul(
                        pw[:, h, :], lhsT=zro[:, 0:P], rhs=zro[:],
                        start=True, stop=True,
                    )

            # upfront: K00 + Q00 interleaved at half-chain granularity so
            # the in-order PE isn't blocked on xh0's second half
            boxK, boxQ = [None], [None]
            boxQ[0] = psP.tile([P, QB], F32, tag="pp", name="pp", bufs=1)

            def q00_part(part):
                ps = boxQ[0]
                h = DT // NPART
                for i in range(h):
                    dt_ = part * h + i
                    nc.tensor.matmul(
                        ps[:],
                        lhsT=wq_sb[:, dt_, 0:P],
                        rhs=xhs[0][:, dt_, :],
                        start=(dt_ == 0),
                        stop=(dt_ == DT - 1),
                    )
                if part == NPART - 1:
                    nc.vector.tensor_scalar_add(
                        out=qT_sb[:, 0, 0:QB], in0=ps[:],
                        scalar1=bq_sb[:, 0:1],
                    )

            for ph in range(2):
                for part in (2 * ph, 2 * ph + 1):
                    qk_part(boxK, wk_sb, bk_sb, kT_sb, xhs[0], 0, 0, part)
                for part in (2 * ph, 2 * ph + 1):
                    q00_part(part)

            # deadline-ordered units (group index when needed, PE cost ns):
            # K(s,jt0) by 4s; K(s,jt1) by 16+4s; V(s,st) by 4s+st+LAG;
            # Q(0,jt1) by 16; Q(qb,*) by 32*qb; xh2/xh3 DMA by g2/g6.
            CH, VCH = 1700, 850
            push(1, 0, lambda: xhs.__setitem__(2, dma_x(2)))
            push(2, 100, emit_bv_bc)
            push(4, 0, lambda: xhs.__setitem__(3, dma_x(3)))
            def push_chain(d, w_sb, b_sb, dstT, s, jt):
                box = [None]
                for part in range(NPART):
                    push(
                        d, 430,
                        lambda part=part: qk_part(
                            box, w_sb, b_sb, dstT, xhs[s], s, jt, part
                        ),
                    )

            for s in range(1, NQB):
                push_chain(4 * s, wk_sb, bk_sb, kT_sb, s, 0)
            for s in range(NQB):
                for st in range(QB // P):
                    push(
                        4 * s + st + LAG,
                        VCH,
                        lambda s=s, st=st: v_chain(xhs[s], s, st),
                    )
            push_chain(KT - 2, wq_sb, bq_sb, qT_sb, 0, 1)
            for s in range(NQB):
                push_chain(KT + 4 * s, wk_sb, bk_sb, kT_sb, s, 1)
            # Q(qb) needed at group GPQ*qb; spread into Act-slack stretches
            qdl = {1: (29, 31), 2: (48, 58), 3: (78, 88)}
            for qb in range(1, NQB):
                for jt in range(JT):
                    push_chain(qdl[qb][jt], wq_sb, bq_sb, qT_sb, qb, jt)

            ledger = [0]  # cumulative filler ns emitted

            def feed(g):
                while epiq:
                    epiq.popleft()()
                while fillers and (
                    fillers[0][0] <= g
                    or (
                        fillers[0][0] <= g + LOOKAHEAD
                        and ledger[0] < (g + 1) * RATE
                    )
                ):
                    item = heapq.heappop(fillers)
                    item[3]()
                    ledger[0] += item[2]

            # attention state
            pend = deque()   # (qb, hg, kt, exps) awaiting attn@V
            ao_tiles = {}    # global qt -> ao_sb tile

            def emit_scores_exp(qb, hg, kt):
                pss = psS.tile([P, 2, QB], F32, tag="pss", name="pss")
                for hp in range(2):
                    nc.tensor.matmul(
                        pss[:, hp, :],
                        lhsT=kT_sb[hp * HD : (hp + 1) * HD, hg, kt * P : (kt + 1) * P],
                        rhs=qT_sb[hp * HD : (hp + 1) * HD, hg, qb * QB : (qb + 1) * QB],
                        start=True,
                        stop=True,
                        tile_position=(hp * HD, 0),
                    )
                exps = expp.tile([P, 2, QB], BF16, tag="exps", name="exps")
                nc.scalar.activation(
                    out=exps[:].rearrange("p a q -> p (a q)"),
                    in_=pss[:].rearrange("p a q -> p (a q)"),
                    func=mybir.ActivationFunctionType.Exp,
                    scale=SCALE,
                )
                return exps

            pav_tiles = {}   # (hg, half) -> pav psum tile for current qb

            def emit_attnv(qb, hg, kt, exps):
                for half in range(2):
                    key = (hg, half)
                    if kt == 0:
                        pav_tiles[key] = psAV.tile(
                            [P, 2, 2, HD + 1], F32, tag=f"pav{half}", bufs=1,
                            name=f"pav{half}",
                        )
                    pav = pav_tiles[key]
                    for qtl2 in range(2):
                        qtl = half * 2 + qtl2
                        for hp in range(2):
                            h = hg * 2 + hp
                            # one accumulation group per PSUM bank: start
                            # zeroes the whole 2KB bank, so only the first
                            # region's kt0 matmul starts, only the last
                            # region's kt15 matmul stops
                            nc.tensor.matmul(
                                pav[:, qtl2, hp, :],
                                lhsT=exps[:, hp, qtl * P : (qtl + 1) * P],
                                rhs=v_sb[
                                    :, kt, h * (HD + 1) : (h + 1) * (HD + 1)
                                ],
                                start=(kt == 0 and qtl2 == 0 and hp == 0),
                                stop=(kt == KT - 1 and qtl2 == 1 and hp == 1),
                                skip_group_check=True,
                            )

            def make_epilogue(qb, hg, half):
                pav = pav_tiles[(hg, half)]
                def emit():
                    rec = recp.tile([P, 2, 2, 1], F32, tag="rec", name="rec")
                    nc.vector.reciprocal(out=rec[:], in_=pav[:, :, :, HD : HD + 1])
                    for qtl2 in range(2):
                        qt = qb * (QB // P) + half * 2 + qtl2
                        if qt not in ao_tiles:
                            ao_tiles[qt] = aop.tile([P, J], F32, tag="ao", name="ao")
                        ao_sb = ao_tiles[qt]
                        for hp in range(2):
                            h = hg * 2 + hp
                            # normalize on the otherwise-idle Pool engine so
                            # the pav WAR release doesn't queue behind DVE
                            nc.vector.tensor_scalar_mul(
                                out=ao_sb[:, h * HD : (h + 1) * HD],
                                in0=pav[:, qtl2, hp, 0:HD],
                                scalar1=rec[:, qtl2, hp, :],
                            )
                        if hg == NHG - 1:
                            # all heads done: queue transpose+Wo fillers,
                            # staggered so units spread across groups
                            # (tight deadlines for the last q-block's tail)
                            qtl = half * 2 + qtl2
                            last = qb == NQB - 1
                            dl = 1 + qtl if last else 3 + 3 * qtl
                            box = [None]
                            push(gcur[0] + dl, 500, trans_unit(box, ao_sb))
                            push(gcur[0] + dl + 1, 450, wo_chunk(box, qt, 0, last))
                            push(gcur[0] + dl + 2, 450, wo_chunk(box, qt, 1, last))
                return emit

            # group loop
            total_groups = NQB * GPQ
            sched = []   # (qb, hg, kt) per group index
            for qb in range(NQB):
                for hg in range(NHG):
                    for kt in range(KT):
                        sched.append((qb, hg, kt))

            gcur = [0]
            for g in range(total_groups + LAG):
                gcur[0] = g
                if g < total_groups:
                    qb, hg, kt = sched[g]
                    exps = emit_scores_exp(qb, hg, kt)
                    pend.append((qb, hg, kt, exps))
                feed(g)
                if g >= LAG and pend:
                    aqb, ahg, akt, aexps = pend.popleft()
                    emit_attnv(aqb, ahg, akt, aexps)
                    if akt == KT - 1:
                        epiq.append(make_epilogue(aqb, ahg, 0))
                        epiq.append(make_epilogue(aqb, ahg, 1))

            if debug:
                nc.sync.dma_start(dbg_q.ap(), qT_sb[:])
                nc.sync.dma_start(dbg_k.ap(), kT_sb[:])
                nc.sync.dma_start(
                    dbg_v.ap()[:, :, 0 : NH * (HD + 1)], v_sb[:]
                )
                for qt in range(8, 16):
                    nc.sync.dma_start(dbg_ao.ap()[qt], ao_tiles[qt][:])

            # drain remaining epilogues and fillers
            while epiq or fillers:
                if epiq:
                    epiq.popleft()()
                else:
                    heapq.heappop(fillers)[3]()

    nc.compile()
    return nc


def _prep_in_maps(inputs, n_cores=8):
    """Per-core input dicts: core c = (batch c//4, head-group c%4)."""
    try:
        import ml_dtypes
        bf16 = ml_dtypes.bfloat16
    except ImportError:
        import jax.numpy as jnp
        bf16 = jnp.bfloat16

    x = np.ascontiguousarray(np.asarray(inputs["inputs"], dtype=np.float32))
    Bb, Ss, Dd = x.shape
    Wq = np.asarray(inputs["Wq"], dtype=np.float32)
    Wk = np.asarray(inputs["Wk"], dtype=np.float32)
    Wv = np.asarray(inputs["Wv"], dtype=np.float32)
    Wo = np.asarray(inputs["Wo"], dtype=np.float32)
    bq = np.asarray(inputs["bq"], dtype=np.float32)
    bk = np.asarray(inputs["bk"], dtype=np.float32)
    bv = np.asarray(inputs["bv"], dtype=np.float32)
    DT = Dd // P
    J = Wq.shape[1] // (n_cores // Bb)
    JT = J // P

    # xT rearranged [128, DT, S] per batch, bf16
    xts = []
    for b in range(Bb):
        xT = x[b].T  # [D, S]
        xts.append(
            np.ascontiguousarray(
                xT.reshape(DT, P, Ss).transpose(1, 0, 2).astype(bf16)
            )
        )

    def wqk_prep(W, sl):
        # [D, J] -> [128, DT, J]
        return np.ascontiguousarray(
            W[:, sl].reshape(DT, P, J).transpose(1, 0, 2).astype(bf16)
        )

    in_maps = []
    for c in range(n_cores):
        b = c // (n_cores // Bb)
        hg4 = c % (n_cores // Bb)
        sl = slice(hg4 * J, (hg4 + 1) * J)
        wo_r = np.ascontiguousarray(
            Wo[sl, :].reshape(JT, P, -1).transpose(1, 0, 2).astype(bf16)
        )
        in_maps.append(
            {
                "xt": xts[b],
                "wq": wqk_prep(Wq, sl),
                "wk": wqk_prep(Wk, sl),
                "wv": wqk_prep(Wv, sl),
                "bq": np.ascontiguousarray(bq[sl].reshape(JT, P).T),
                "bk": np.ascontiguousarray(bk[sl].reshape(JT, P).T),
                "bv": np.ascontiguousarray(bv[sl].reshape(1, J).astype(bf16)),
                "wo": wo_r,
            }
        )
    return in_maps


_NC_CACHE = {}


def kernel(**inputs) -> np.ndarray:
    from concourse.bass_utils import run_bass_kernel_spmd

    try:
        import jax

        jax.config.update("jax_compilation_cache_dir", "/tmp/jaxcache")
    except Exception:
        pass

    x = np.asarray(inputs["inputs"])
    Bb, Ss, Dd = x.shape
    DOUT = np.asarray(inputs["Wo"]).shape[1]

    key = (Bb, Ss, Dd, DOUT)
    if key not in _NC_CACHE:
        _NC_CACHE[key] = build_nc(S=Ss, D=Dd, DOUT=DOUT)
    nc = _NC_CACHE[key]

    in_maps = _prep_in_maps(inputs, n_cores=8)
    res = None
    for attempt in range(3):
        try:
            res = run_bass_kernel_spmd(nc, in_maps, core_ids=list(range(8)))
            break
        except Exception:
            if attempt == 2:
                raise
            import time

            time.sleep(5)
    gpb = 8 // Bb  # cores per batch
    outs = []
    for b in range(Bb):
        acc = np.zeros((Ss, DOUT), dtype=np.float64)
        for g in range(gpb):
            acc += np.asarray(res.results[b * gpb + g]["out"], dtype=np.float64)
        outs.append(acc.astype(np.float32))
    out = np.stack(outs, axis=0)
    out = out + np.asarray(inputs["bo"], dtype=np.float32)[None, None, :]
    return out


# revision 3
# speedup vs baseline: 1.0067x; 1.0067x over previous
"""Multi-head attention Trainium2 kernel (8-core batch x head-group parallel).

Problem: B=2, S=2048, D=1024, H=16 heads x HD=64.
Sharding: core c owns batch c//4 and head-group c%4 (4 heads, J=256 columns
of Wq/Wk/Wv, 256 rows of Wo). Each core produces a partial output (its
heads' contribution through Wo); the host sums 4 partials per batch + bo.

All matmul operands are bf16 (flat 1 cycle/row on the PE; fp32r pays 4x
below N=256). PSUM accumulation stays fp32.

Per-core compute (S=2048, 4 heads as 2 head-pairs hg in {0,1}):
  K^T/Q^T  = W^T @ X^T          [128(2-head hd), 2048]  (jt = head-pair)
  V        = X @ Wv             [s, 256] orientation -> v_sb[k-tile, 4*65]
                                (65th column per head = ones, so attn@V's
                                 output column 64 = softmax denominator)
  S^T[k,q] = K_tile^T Q         head-pairs row-stacked in the PE
                                (tile_position h*64)
  P        = exp(S^T / 8)       ScalarE, scale folded in; max-subtraction
                                skipped (scores ~N(0,1))
  ao[q,hd] = sum_k P^T V        lhsT = exp tile [128k,128q] (stationary),
                                rhs = v_sb [128k, 65]: N=65 per matmul --
                                full PE-column utilization vs streaming q
  normalize: reciprocal of col 64, per-partition tensor_scalar mult (DVE)
  aoT      = transpose(ao)      fp32 PE transposes, then bf16 copy
  partial  = aoT^T @ Wo         contraction over the core's 256 head dims

Scheduling: Tile's per-engine order follows emission order. The ScalarE
exp stream (128 x [128,1024] activations, ~1.04us each) is the pacing
engine; PE busy is slightly above it, so everything aims to keep exp fed:
scores groups (hg, kt) stream on the PE with exp one group behind and
attn@V LAG groups behind (large lag = slack for filler spikes; the
single-buffered pav psum slots still rotate since epilogues release them
via one fast pav->sbuf copy). Projection chains (split into 2-matmul
quarter units) and output (transpose / Wo-chunk) units are deadline-
scheduled fillers fed at a uniform target rate between groups. DMAs are
issued on the sync queue in need-order (wk, wq and the first x quarters
first; wo last via gpsimd); dummy matmuls burn the PE p-state ramp while
the first DMAs land.

PSUM accumulation groups NEVER share a 2KB bank (start_tensor_calc zeroes
the whole bank): scores 2x2 banks, pav 1 bank per (head-pair half) with a
single start/stop for its four [128,65] regions, chain scratch 1 (ppc),
shared V/transpose/Wo scratch 1 (pp).
"""

import numpy as np

import concourse.bass as bass
import concourse.bacc as bacc
import concourse.tile as tile
import concourse.mybir as mybir
from concourse.masks import make_identity

F32 = mybir.dt.float32
BF16 = mybir.dt.bfloat16

P = 128


def build_nc(
    S=2048,          # sequence length (one batch per core)
    D=1024,          # model dim
    DOUT=1024,       # output dim (cols of Wo)
    J=256,           # head-dim columns per core (4 heads x 64)
    QB=512,          # q-block
    LAG=44,          # attn@V lag behind scores/exp, in groups
    exp_bufs=48,
    RATE=700,        # target filler ns per group (debt feeding)
    LOOKAHEAD=3,
    debug=False,
    prolog_feed=2,   # fillers per group while projections remain
    steady_feed=1,
):
    HD = 64
    NH = J // HD         # heads per core (4)
    NHG = NH // 2        # head-pairs (2)
    JT = J // P          # 128-wide jt chunks (2) == head-pairs
    DT = D // P          # contraction tiles for projections (8)
    NQB = S // QB        # q-blocks (4)
    KT = S // P          # key tiles (16)
    NQT = S // P         # q tiles (16)
    SCALE = 1.0 / float(np.sqrt(HD))
    assert J == 2 * P and NHG == JT
    GPQ = NHG * KT       # groups per q-block (32)

    nc = bacc.Bacc(None, target_bir_lowering=False)

    # host pre-rearranged inputs (see _prep_in_maps)
    xt_h = nc.dram_tensor("xt", [P, DT, S], BF16, kind="ExternalInput")
    wq_h = nc.dram_tensor("wq", [P, DT, J], BF16, kind="ExternalInput")
    wk_h = nc.dram_tensor("wk", [P, DT, J], BF16, kind="ExternalInput")
    wv_h = nc.dram_tensor("wv", [P, DT, J], BF16, kind="ExternalInput")
    bq_h = nc.dram_tensor("bq", [P, JT], F32, kind="ExternalInput")
    bk_h = nc.dram_tensor("bk", [P, JT], F32, kind="ExternalInput")
    bv_h = nc.dram_tensor("bv", [1, J], BF16, kind="ExternalInput")
    wo_h = nc.dram_tensor("wo", [P, JT, DOUT], BF16, kind="ExternalInput")
    out_h = nc.dram_tensor("out", [S, DOUT], F32, kind="ExternalOutput")
    if debug:
        dbg_q = nc.dram_tensor("dbg_q", [P, JT, S], BF16, kind="ExternalOutput")
        dbg_k = nc.dram_tensor("dbg_k", [P, JT, S], BF16, kind="ExternalOutput")
        dbg_v = nc.dram_tensor("dbg_v", [P, KT, J + 4], BF16, kind="ExternalOutput")
        dbg_ao = nc.dram_tensor("dbg_ao", [16, P, J], F32, kind="ExternalOutput")

    with tile.TileContext(nc) as tc:
        with (
            tc.tile_pool(name="const", bufs=1) as const,
            tc.tile_pool(name="xin", bufs=4) as xin,
            tc.tile_pool(name="expp", bufs=exp_bufs) as expp,
            tc.tile_pool(name="aop", bufs=8) as aop,
            tc.tile_pool(name="aotp", bufs=3) as aotp,
            tc.tile_pool(name="recp", bufs=2) as recp,
            tc.tile_pool(name="outp", bufs=6) as outp,
            tc.tile_pool(name="psS", bufs=2, space="PSUM") as psS,
            tc.tile_pool(name="psAV", bufs=1, space="PSUM") as psAV,
            tc.tile_pool(name="psP", bufs=2, space="PSUM") as psP,
        ):
            # ---- constants / weights (DMA order = need order: wk + biases
            # + first x half gate the first matmul; wo is needed last) ----
            wq_sb = const.tile([P, DT, J], BF16)
            wk_sb = const.tile([P, DT, J], BF16)
            wv_sb = const.tile([P, DT, J], BF16)
            wo_sb = const.tile([P, JT, DOUT], BF16)
            bq_sb = const.tile([P, JT], F32)
            bk_sb = const.tile([P, JT], F32)
            bv_row = const.tile([1, J], BF16)
            # all early-needed DMAs go on the sync queue in need-order
            # (single FIFO => device serves them in this order); only the
            # late-needed wo rides the slow gpsimd/SWDGE path


            ident_f = const.tile([P, P], F32)
            make_identity(nc, ident_f[:])
            ones_f = const.tile([P, 1], F32)
            nc.vector.memset(ones_f[:], 1.0)
            ones1_b = const.tile([1, P], BF16)
            nc.vector.tensor_copy(
                out=ones1_b[:], in_=ones_f[0:1, :].to_broadcast((1, P))
            )
            bv_bc = const.tile([P, J], BF16)

            def emit_bv_bc():
                # broadcast bv to all partitions via K=1 matmul: [128, J] bf16
                pbv = psP.tile([P, 512], F32, tag="pp", name="pp", bufs=1)
                nc.tensor.matmul(
                    pbv[:, 0:J], lhsT=ones1_b[:], rhs=bv_row[:],
                    start=True, stop=True,
                )
                nc.vector.tensor_copy(out=bv_bc[:], in_=pbv[:, 0:J])

            # ---- persistent projection outputs ----
            qT_sb = const.tile([P, JT, S], BF16)   # [hd-pair rows, jt, q]
            kT_sb = const.tile([P, JT, S], BF16)
            # v with ones columns: [k-part, kt, 4 heads x 65]
            v_sb = const.tile([P, KT, NH * (HD + 1)], BF16)
            # fill the ones columns once (static)
            ones_view = bass.AP(
                tensor=v_sb.tensor,
                offset=v_sb.offset + HD,  # first ones col at 64
                ap=[v_sb.ap[0], [NH * (HD + 1), KT], [HD + 1, NH], [1, 1]],
            )
            nc.vector.tensor_copy(
                out=ones_view,
                in_=ones_f[:].unsqueeze(1).to_broadcast((P, KT, NH, 1)),
            )

            # ---------------- projection filler units ----------------
            def dma_x(sblk):
                # two half-DMAs so the first chain can start sooner
                xh = xin.tile([P, DT, QB], BF16, tag="xh", name="xh")
                for dh in range(2):
                    w = DT // 2
                    nc.sync.dma_start(
                        xh[:, dh * w : (dh + 1) * w, :],
                        xt_h.ap()[
                            :, dh * w : (dh + 1) * w, sblk * QB : (sblk + 1) * QB
                        ],
                    )
                return xh

            NPART = 4  # chain split granularity (2 matmuls per part)

            def qk_part(box, w_sb, b_sb, dstT, xh, sblk, jt, part):
                """1/NPART of a projection chain. part 0 opens the psum
                accumulation (own tag so interleaved units can't cycle),
                the last part closes it + bias-copy."""
                if part == 0:
                    box[0] = psP.tile([P, QB], F32, tag="ppc", name="ppc", bufs=1)
                ps = box[0]
                h = DT // NPART
                for i in range(h):
                    dt_ = part * h + i
                    nc.tensor.matmul(
                        ps[:],
                        lhsT=w_sb[:, dt_, jt * P : (jt + 1) * P],
                        rhs=xh[:, dt_, :],
                        start=(dt_ == 0),
                        stop=(dt_ == DT - 1),
                    )
                if part == NPART - 1:
                    nc.vector.tensor_scalar_add(
                        out=dstT[:, jt, sblk * QB : (sblk + 1) * QB],
                        in0=ps[:],
                        scalar1=b_sb[:, jt : jt + 1],
                    )

            def qk_chain(w_sb, b_sb, dstT, xh, sblk, jt):
                box = [None]
                for part in range(NPART):
                    qk_part(box, w_sb, b_sb, dstT, xh, sblk, jt, part)

            def v_chain(xh, sblk, st):
                """V projection for one 128-seq subtile, [s, j] orientation."""
                kt = sblk * (QB // P) + st
                ps = psP.tile([P, QB], F32, tag="pp", name="pp", bufs=1)
                for dt_ in range(DT):
                    nc.tensor.matmul(
                        ps[:, 0:J],
                        lhsT=xh[:, dt_, st * P : (st + 1) * P],
                        rhs=wv_sb[:, dt_, :],
                        start=(dt_ == 0),
                        stop=(dt_ == DT - 1),
                    )
                # bias add + copy into v_sb (strided dst skips ones columns)
                vdst = bass.AP(
                    tensor=v_sb.tensor,
                    offset=v_sb.offset + kt * NH * (HD + 1),
                    ap=[v_sb.ap[0], [HD + 1, NH], [1, HD]],
                )
                nc.vector.tensor_tensor(
                    out=vdst,
                    in0=ps[:, 0:J].rearrange("p (h d) -> p h d", h=NH),
                    in1=bv_bc[:].rearrange("p (h d) -> p h d", h=NH),
                    op=mybir.AluOpType.add,
                )



            # ---------------- output-side filler units ----------------
            def trans_unit(box, ao_sb):
                """Transpose ao [128q, 256] -> aoT bf16 (into box[0])."""
                def emit():
                    aoT = aotp.tile([P, JT, P], BF16, tag="aoT", name="aoT")
                    box[0] = aoT
                    for jt in range(JT):
                        pt = psP.tile([P, QB], F32, tag="pp", name="pp", bufs=1)
                        nc.tensor.transpose(
                            pt[:, 0:P], ao_sb[:, jt * P : (jt + 1) * P], ident_f[:]
                        )
                        nc.vector.tensor_copy(out=aoT[:, jt, :], in_=pt[:, 0:P])
                return emit

            def wo_chunk(box, qt, ch, last):
                """One Wo chunk for global q-tile qt. In the tail (last
                q-block) the psum comes from the then-free scores pool (4
                banks => deeper pipelining) and the psum->sbuf copy goes to
                the then-idle ScalarE instead of DVE."""
                def emit():
                    aoT = box[0]
                    po = (
                        psS.tile([P, 2, QB], F32, tag="pss", name="pss")[:, 0, :]
                        if last
                        else psP.tile([P, QB], F32, tag="pp", name="pp", bufs=1)[:]
                    )
                    for jt in range(JT):
                        nc.tensor.matmul(
                            po,
                            lhsT=aoT[:, jt, :],
                            rhs=wo_sb[:, jt, ch * 512 : (ch + 1) * 512],
                            start=(jt == 0),
                            stop=(jt == JT - 1),
                        )
                    o_sb = outp.tile([P, 512], F32, tag="o_sb", name="o_sb")
                    if last:
                        nc.scalar.activation(
                            out=o_sb[:],
                            in_=po,
                            func=mybir.ActivationFunctionType.Copy,
                        )
                    else:
                        nc.vector.tensor_copy(out=o_sb[:], in_=po)
                    nc.sync.dma_start(
                        out_h.ap()[
                            qt * P : (qt + 1) * P, ch * 512 : (ch + 1) * 512
                        ],
                        o_sb[:],
                    )
                return emit

            # ---------------- main emission ----------------
            from collections import deque
            import heapq

            fillers = []  # heap of (deadline_group, seq, pe_cost_ns, fn)
            epiq = deque()
            fseq = [0]

            def push(deadline, cost, fn):
                heapq.heappush(fillers, (deadline, fseq[0], cost, fn))
                fseq[0] += 1

            # DMA need-order on the single sync FIFO: xh0a, wk, wq, biases,
            # xh0b, xh1, wv, bv; wo via gpsimd; xh2/xh3 deferred to fillers
            # so they can't jump ahead of early weights.
            xhs = [None] * NQB
            xh0 = xin.tile([P, DT, QB], BF16, tag="xh", name="xh")
            xhs[0] = xh0
            qw = DT // 4
            nc.sync.dma_start(xh0[:, 0:qw, :], xt_h.ap()[:, 0:qw, 0:QB])
            nc.sync.dma_start(wk_sb[:], wk_h.ap())
            nc.sync.dma_start(wq_sb[:], wq_h.ap())
            for i in range(1, 4):
                nc.sync.dma_start(
                    xh0[:, i * qw : (i + 1) * qw, :],
                    xt_h.ap()[:, i * qw : (i + 1) * qw, 0:QB],
                )
            nc.sync.dma_start(bk_sb[:], bk_h.ap())
            nc.sync.dma_start(bq_sb[:], bq_h.ap())
            xhs[1] = dma_x(1)
            nc.sync.dma_start(wv_sb[:], wv_h.ap())
            nc.sync.dma_start(bv_row[:], bv_h.ap())
            nc.gpsimd.dma_start(wo_sb[:], wo_h.ap())

            # PE warm-up: burn the p-state ramp on dummy matmuls while the
            # DMAs land (results unused; input is a zeroed const tile)
            zro = const.tile([P, 512], BF16)
            nc.vector.memset(zro[:], 0.0)
            for _ in range(6):
                pw = psS.tile([P, 2, QB], F32, tag="pss", name="pss")
                for h in range(2):
                    nc.tensor.matmul(
                        pw[:, h, :], lhsT=zro[:, 0:P], rhs=zro[:],
                        start=True, stop=True,
                    )

            # upfront: K00 + Q00 interleaved at half-chain granularity so
            # the in-order PE isn't blocked on xh0's second half
            boxK, boxQ = [None], [None]
            boxQ[0] = psP.tile([P, QB], F32, tag="pp", name="pp", bufs=1)

            def q00_part(part):
                ps = boxQ[0]
                h = DT // NPART
                for i in range(h):
                    dt_ = part * h + i
                    nc.tensor.matmul(
                        ps[:],
                        lhsT=wq_sb[:, dt_, 0:P],
                        rhs=xhs[0][:, dt_, :],
                        start=(dt_ == 0),
                        stop=(dt_ == DT - 1),
                    )
                if part == NPART - 1:
                    nc.vector.tensor_scalar_add(
                        out=qT_sb[:, 0, 0:QB], in0=ps[:],
                        scalar1=bq_sb[:, 0:1],
                    )

            for part in range(NPART):
                qk_part(boxK, wk_sb, bk_sb, kT_sb, xhs[0], 0, 0, part)
                q00_part(part)

            # deadline-ordered units (group index when needed, PE cost ns):
            # K(s,jt0) by 4s; K(s,jt1) by 16+4s; V(s,st) by 4s+st+LAG;
            # Q(0,jt1) by 16; Q(qb,*) by 32*qb; xh2/xh3 DMA by g2/g6.
            CH, VCH = 1700, 850
            push(1, 0, lambda: xhs.__setitem__(2, dma_x(2)))
            push(2, 100, emit_bv_bc)
            push(4, 0, lambda: xhs.__setitem__(3, dma_x(3)))
            def push_chain(d, w_sb, b_sb, dstT, s, jt):
                box = [None]
                for part in range(NPART):
                    push(
                        d, 430,
                        lambda part=part: qk_part(
                            box, w_sb, b_sb, dstT, xhs[s], s, jt, part
                        ),
                    )

            for s in range(1, NQB):
                push_chain(4 * s, wk_sb, bk_sb, kT_sb, s, 0)
            for s in range(NQB):
                for st in range(QB // P):
                    push(
                        4 * s + st + LAG,
                        VCH,
                        lambda s=s, st=st: v_chain(xhs[s], s, st),
                    )
            push_chain(KT - 2, wq_sb, bq_sb, qT_sb, 0, 1)
            for s in range(NQB):
                push_chain(KT + 4 * s, wk_sb, bk_sb, kT_sb, s, 1)
            # Q(qb) needed at group GPQ*qb; spread into Act-slack stretches
            qdl = {1: (29, 31), 2: (48, 58), 3: (78, 88)}
            for qb in range(1, NQB):
                for jt in range(JT):
                    push_chain(qdl[qb][jt], wq_sb, bq_sb, qT_sb, qb, jt)

            ledger = [0]  # cumulative filler ns emitted

            def feed(g):
                while epiq:
                    epiq.popleft()()
                while fillers and (
                    fillers[0][0] <= g
                    or (
                        fillers[0][0] <= g + LOOKAHEAD
                        and ledger[0] < (g + 1) * RATE
                    )
                ):
                    item = heapq.heappop(fillers)
                    item[3]()
                    ledger[0] += item[2]

            # attention state
            pend = deque()   # (qb, hg, kt, exps) awaiting attn@V
            ao_tiles = {}    # global qt -> ao_sb tile

            def emit_scores_exp(qb, hg, kt):
                pss = psS.tile([P, 2, QB], F32, tag="pss", name="pss")
                for hp in range(2):
                    nc.tensor.matmul(
                        pss[:, hp, :],
                        lhsT=kT_sb[hp * HD : (hp + 1) * HD, hg, kt * P : (kt + 1) * P],
                        rhs=qT_sb[hp * HD : (hp + 1) * HD, hg, qb * QB : (qb + 1) * QB],
                        start=True,
                        stop=True,
                        tile_position=(hp * HD, 0),
                    )
                exps = expp.tile([P, 2, QB], BF16, tag="exps", name="exps")
                nc.scalar.activation(
                    out=exps[:].rearrange("p a q -> p (a q)"),
                    in_=pss[:].rearrange("p a q -> p (a q)"),
                    func=mybir.ActivationFunctionType.Exp,
                    scale=SCALE,
                )
                return exps

            pav_tiles = {}   # (hg, half) -> pav psum tile for current qb

            def emit_attnv(qb, hg, kt, exps):
                for half in range(2):
                    key = (hg, half)
                    if kt == 0:
                        pav_tiles[key] = psAV.tile(
                            [P, 2, 2, HD + 1], F32, tag=f"pav{half}", bufs=1,
                            name=f"pav{half}",
                        )
                    pav = pav_tiles[key]
                    for qtl2 in range(2):
                        qtl = half * 2 + qtl2
                        for hp in range(2):
                            h = hg * 2 + hp
                            # one accumulation group per PSUM bank: start
                            # zeroes the whole 2KB bank, so only the first
                            # region's kt0 matmul starts, only the last
                            # region's kt15 matmul stops
                            nc.tensor.matmul(
                                pav[:, qtl2, hp, :],
                                lhsT=exps[:, hp, qtl * P : (qtl + 1) * P],
                                rhs=v_sb[
                                    :, kt, h * (HD + 1) : (h + 1) * (HD + 1)
                                ],
                                start=(kt == 0 and qtl2 == 0 and hp == 0),
                                stop=(kt == KT - 1 and qtl2 == 1 and hp == 1),
                                skip_group_check=True,
                            )

            def make_epilogue(qb, hg, half):
                pav = pav_tiles[(hg, half)]
                def emit():
                    # single copy releases the pav psum bank fast (the WAR
                    # gates the next head-pair's attn@V)
                    pcop = recp.tile([P, 2, 2, HD + 1], F32, tag="pcop", name="pcop")
                    nc.vector.tensor_copy(out=pcop[:], in_=pav[:])
                    rec = recp.tile([P, 2, 2, 1], F32, tag="rec", name="rec")
                    nc.vector.reciprocal(out=rec[:], in_=pcop[:, :, :, HD : HD + 1])
                    for qtl2 in range(2):
                        qt = qb * (QB // P) + half * 2 + qtl2
                        if qt not in ao_tiles:
                            ao_tiles[qt] = aop.tile([P, J], F32, tag="ao", name="ao")
                        ao_sb = ao_tiles[qt]
                        for hp in range(2):
                            h = hg * 2 + hp
                            # normalize on the otherwise-idle Pool engine so
                            # the pav WAR release doesn't queue behind DVE
                            nc.vector.tensor_scalar_mul(
                                out=ao_sb[:, h * HD : (h + 1) * HD],
                                in0=pcop[:, qtl2, hp, 0:HD],
                                scalar1=rec[:, qtl2, hp, :],
                            )
                        if hg == NHG - 1:
                            # all heads done: queue transpose+Wo fillers,
                            # staggered so units spread across groups
                            # (tight deadlines for the last q-block's tail)
                            qtl = half * 2 + qtl2
                            last = qb == NQB - 1
                            dl = 1 + qtl if last else 3 + 3 * qtl
                            box = [None]
                            push(gcur[0] + dl, 500, trans_unit(box, ao_sb))
                            push(gcur[0] + dl + 1, 450, wo_chunk(box, qt, 0, last))
                            push(gcur[0] + dl + 2, 450, wo_chunk(box, qt, 1, last))
                return emit

            # group loop
            total_groups = NQB * GPQ
            sched = []   # (qb, hg, kt) per group index
            for qb in range(NQB):
                for hg in range(NHG):
                    for kt in range(KT):
                        sched.append((qb, hg, kt))

            gcur = [0]
            for g in range(total_groups + LAG):
                gcur[0] = g
                if g < total_groups:
                    qb, hg, kt = sched[g]
                    exps = emit_scores_exp(qb, hg, kt)
                    pend.append((qb, hg, kt, exps))
                feed(g)
                if g >= LAG and pend:
                    aqb, ahg, akt, aexps = pend.popleft()
                    emit_attnv(aqb, ahg, akt, aexps)
                    if akt == KT - 1:
                        epiq.append(make_epilogue(aqb, ahg, 0))
                        epiq.append(make_epilogue(aqb, ahg, 1))

            if debug:
                nc.sync.dma_start(dbg_q.ap(), qT_sb[:])
                nc.sync.dma_start(dbg_k.ap(), kT_sb[:])
                nc.sync.dma_start(
                    dbg_v.ap()[:, :, 0 : NH * (HD + 1)], v_sb[:]
                )
                for qt in range(8, 16):
                    nc.sync.dma_start(dbg_ao.ap()[qt], ao_tiles[qt][:])

            # drain remaining epilogues and fillers
            while epiq or fillers:
                if epiq:
                    epiq.popleft()()
                else:
                    heapq.heappop(fillers)[3]()

    nc.compile()
    return nc


def _prep_in_maps(inputs, n_cores=8):
    """Per-core input dicts: core c = (batch c//4, head-group c%4)."""
    try:
        import ml_dtypes
        bf16 = ml_dtypes.bfloat16
    except ImportError:
        import jax.numpy as jnp
        bf16 = jnp.bfloat16

    x = np.ascontiguousarray(np.asarray(inputs["inputs"], dtype=np.float32))
    Bb, Ss, Dd = x.shape
    Wq = np.asarray(inputs["Wq"], dtype=np.float32)
    Wk = np.asarray(inputs["Wk"], dtype=np.float32)
    Wv = np.asarray(inputs["Wv"], dtype=np.float32)
    Wo = np.asarray(inputs["Wo"], dtype=np.float32)
    bq = np.asarray(inputs["bq"], dtype=np.float32)
    bk = np.asarray(inputs["bk"], dtype=np.float32)
    bv = np.asarray(inputs["bv"], dtype=np.float32)
    DT = Dd // P
    J = Wq.shape[1] // (n_cores // Bb)
    JT = J // P

    # xT rearranged [128, DT, S] per batch, bf16
    xts = []
    for b in range(Bb):
        xT = x[b].T  # [D, S]
        xts.append(
            np.ascontiguousarray(
                xT.reshape(DT, P, Ss).transpose(1, 0, 2).astype(bf16)
            )
        )

    def wqk_prep(W, sl):
        # [D, J] -> [128, DT, J]
        return np.ascontiguousarray(
            W[:, sl].reshape(DT, P, J).transpose(1, 0, 2).astype(bf16)
        )

    in_maps = []
    for c in range(n_cores):
        b = c // (n_cores // Bb)
        hg4 = c % (n_cores // Bb)
        sl = slice(hg4 * J, (hg4 + 1) * J)
        wo_r = np.ascontiguousarray(
            Wo[sl, :].reshape(JT, P, -1).transpose(1, 0, 2).astype(bf16)
        )
        in_maps.append(
            {
                "xt": xts[b],
                "wq": wqk_prep(Wq, sl),
                "wk": wqk_prep(Wk, sl),
                "wv": wqk_prep(Wv, sl),
                "bq": np.ascontiguousarray(bq[sl].reshape(JT, P).T),
                "bk": np.ascontiguousarray(bk[sl].reshape(JT, P).T),
                "bv": np.ascontiguousarray(bv[sl].reshape(1, J).astype(bf16)),
                "wo": wo_r,
            }
        )
    return in_maps


_NC_CACHE = {}


def kernel(**inputs) -> np.ndarray:
    from concourse.bass_utils import run_bass_kernel_spmd

    try:
        import jax

        jax.config.update("jax_compilation_cache_dir", "/tmp/jaxcache")
    except Exception:
        pass

    x = np.asarray(inputs["inputs"])
    Bb, Ss, Dd = x.shape
    DOUT = np.asarray(inputs["Wo"]).shape[1]

    key = (Bb, Ss, Dd, DOUT)
    if key not in _NC_CACHE:
        _NC_CACHE[key] = build_nc(S=Ss, D=Dd, DOUT=DOUT)
    nc = _NC_CACHE[key]

    in_maps = _prep_in_maps(inputs, n_cores=8)
    res = None
    for attempt in range(3):
        try:
            res = run_bass_kernel_spmd(nc, in_maps, core_ids=list(range(8)))
            break
        except Exception:
            if attempt == 2:
                raise
            import time

            time.sleep(5)
    gpb = 8 // Bb  # cores per batch
    outs = []
    for b in range(Bb):
        acc = np.zeros((Ss, DOUT), dtype=np.float64)
        for g in range(gpb):
            acc += np.asarray(res.results[b * gpb + g]["out"], dtype=np.float64)
        outs.append(acc.astype(np.float32))
    out = np.stack(outs, axis=0)
    out = out + np.asarray(inputs["bo"], dtype=np.float32)[None, None, :]
    return out
